# revision 15
# baseline (speedup 1.0000x reference)
"""Trainium2 Bass kernel for the CO2RR message-passing GNN.

Strategy (8 NeuronCores, SPMD single NEFF):
  - Nodes sharded 5000/core (padded to 5120 = 40 blocks x 128); edges sharded
    by dst-node owner and grouped into the owner's 128-node dst blocks.
  - Full node-state h (fp32) is replicated via an AllGather after every layer;
    per-edge h[src] rows are fetched with batched indirect DMA gathers
    (one descriptor per 512B row, 128*T rows per call).
  - The edge filter MLP runs feature-major (transposed) on the TensorE with
    bf16 weights; the last MLP layer flips orientation (edge-major) so the
    scatter-sum can run as one-hot matmuls accumulating dst blocks in PSUM.
  - The node update MLP + LayerNorm + residual and the two readout heads run
    per node shard; per-graph sums are produced as one-hot matmul partials and
    combined on the host (tiny [128,8] arrays).
"""

import numpy as np
import ml_dtypes

import concourse.bass as bass
import concourse.tile as tile
import concourse.mybir as mybir
from concourse.bass_utils import run_bass_kernel_spmd

F32 = mybir.dt.float32
BF16 = mybir.dt.bfloat16
I32 = mybir.dt.int32

NCORES = 8
HID = 128
NRBF = 64
CUTOFF = 6.0
GAMMA = 10.0
LN_EPS = 1e-5
PAD_SLOT = 300.0

# pool buffer depths (module-level so experiments can override)
BUFS1 = 2
BUFS2 = 2
BUFS3 = 3
BUFS4 = 4
PBUFS3 = 3
PBUFS2 = 2
PBUFS2A = 2
DEBUG_TAPS = False
LAYER_LIMIT = 99
FAKE_CC = False  # replace collectives with local DMA (TimelineSim timing builds)
ABL_GATHER = False   # ablation: static DMA instead of indirect gathers
ABL_MLP = False      # ablation: skip filter-MLP matmuls/evacs
ABL_SCATTER = False  # ablation: skip onehot+scatter
ABL_NODE = False     # ablation: skip node phase


# --------------------------------------------------------------------------
# workaround: this walrus build only supports ONE sync-wait per instruction.
def _split_multi_waits(nc):
    n_split = 0
    for f in nc.m.functions:
        for bb in f.blocks:
            lst = bb.instructions
            i = 0
            while i < len(lst):
                inst = lst[i]
                si = inst.sync_info
                waits = list(si.on_wait) if (si is not None and si.on_wait) else []
                if len(waits) > 1:
                    carriers = []
                    for w in waits[:-1]:
                        carriers.append(mybir.InstNoOp(
                            name=nc.get_next_instruction_name(),
                            sync_info=mybir.SyncInfo(on_wait=[w], on_update=[]),
                            bass_nofuse=True,
                            engine=inst.engine,
                        ))
                    inst.sync_info = mybir.SyncInfo(
                        on_wait=[waits[-1]],
                        on_update=list(si.on_update) if si.on_update else [],
                    )
                    for j, nop in enumerate(carriers):
                        lst.insert(i + j, nop)
                    i += len(carriers)
                    n_split += 1
                i += 1
    return n_split


# --------------------------------------------------------------------------
# host-side prep: shard nodes/edges, build gather-index + one-hot-slot layouts
def _prep(inputs):
    src = np.asarray(inputs['edge_index'][0]).astype(np.int64)
    dst = np.asarray(inputs['edge_index'][1]).astype(np.int64)
    d = np.asarray(inputs['edge_dist'], np.float32)[:, 0]
    batch = np.asarray(inputs['batch']).astype(np.int64)
    n_atoms = src.shape[0] and np.asarray(inputs['atomic_numbers']).shape[0]
    n_edges = src.shape[0]
    n_batch = int(inputs['batch_size'])

    npc_real = n_atoms // NCORES
    assert npc_real * NCORES == n_atoms
    npc = ((npc_real + 127) // 128) * 128
    nblk = npc // 128

    def pad_id(n):
        return (n // npc_real) * npc + (n % npc_real)

    core_of = dst // npc_real
    blk_of = (dst % npc_real) // 128
    slot_of = (dst % npc_real) % 128

    counts = np.zeros((NCORES, nblk), np.int64)
    np.add.at(counts, (core_of, blk_of), 1)
    T = np.maximum(1, ((counts.max(axis=0) + 127) // 128)).astype(np.int64)
    offs = np.zeros(nblk + 1, np.int64)
    offs[1:] = np.cumsum(T)
    NT = int(offs[-1])
    e_core = NT * 128

    hidx = np.zeros((NCORES, e_core), np.int32)
    dstloc = np.full((NCORES, e_core), PAD_SLOT, np.float32)
    dpad = np.zeros((NCORES, e_core), np.float32)

    order = np.lexsort((blk_of, core_of))
    ptr = 0
    for c in range(NCORES):
        for b in range(nblk):
            n = counts[c, b]
            sel = order[ptr:ptr + n]
            base = offs[b] * 128
            hidx[c, base:base + n] = pad_id(src[sel])
            dstloc[c, base:base + n] = slot_of[sel]
            dpad[c, base:base + n] = d[sel]
            ptr += n
    assert ptr == n_edges

    # per-call gather layout: idx[p, j] = row for edge j*128+p of the block
    hidx_l = np.zeros((NCORES, 128, NT), np.int32)
    dstloc_l = np.zeros((NCORES, 128, NT), np.float32)
    for b in range(nblk):
        seg = slice(offs[b] * 128, offs[b + 1] * 128)
        tb = int(T[b])
        hidx_l[:, :, offs[b]:offs[b + 1]] = \
            hidx[:, seg].reshape(NCORES, tb, 128).transpose(0, 2, 1)
        dstloc_l[:, :, offs[b]:offs[b + 1]] = \
            dstloc[:, seg].reshape(NCORES, tb, 128).transpose(0, 2, 1)

    cf = 0.5 * (np.cos(np.pi * dpad / CUTOFF) + 1.0)
    lncf = np.log(np.maximum(cf, 1e-30)).clip(-60.0)
    rbfrhs = np.stack([-GAMMA * dpad * dpad, dpad,
                       np.ones_like(dpad), lncf], axis=1).astype(np.float32)

    centers = np.linspace(0.0, CUTOFF, NRBF, dtype=np.float32)
    rbf_lhsT = np.stack([np.ones(NRBF, np.float32), 2 * GAMMA * centers,
                         -GAMMA * centers * centers,
                         np.ones(NRBF, np.float32)], 0)

    cnt = np.zeros(n_atoms, np.float32)
    np.add.at(cnt, dst, 1.0)
    cnt = np.maximum(cnt, 1.0)
    invcnt = np.zeros((NCORES, 128, nblk), np.float32)
    nfT = np.zeros((NCORES, NRBF, npc), np.float32)
    an_idx = np.zeros((NCORES, 128, nblk), np.int32)
    bslot = np.full((NCORES, 128, nblk), PAD_SLOT, np.float32)
    bbase = np.zeros(NCORES, np.int64)
    nf = np.asarray(inputs['node_features'], np.float32)
    an = np.asarray(inputs['atomic_numbers']).astype(np.int64)
    for c in range(NCORES):
        r = slice(c * npc_real, (c + 1) * npc_real)
        padded = np.concatenate([1.0 / cnt[r], np.ones(npc - npc_real, np.float32)])
        invcnt[c] = padded.reshape(nblk, 128).T
        nfT[c, :, :npc_real] = nf[r].T
        an_idx[c] = np.concatenate(
            [an[r], np.zeros(npc - npc_real, np.int64)]).reshape(nblk, 128).T
        bc = batch[r]
        bbase[c] = int(bc.min())
        assert bc.max() - bc.min() < 128, "batch span per shard exceeds 128"
        bslot[c] = np.concatenate(
            [(bc - bbase[c]).astype(np.float32),
             np.full(npc - npc_real, PAD_SLOT, np.float32)]).reshape(nblk, 128).T

    bcnt = np.bincount(batch, minlength=n_batch).astype(np.float32)
    iota = np.broadcast_to(np.arange(128, dtype=np.float32)[None, :],
                           (128, 128)).copy()

    return dict(n_atoms=n_atoms, n_batch=n_batch, npc_real=npc_real, npc=npc,
                nblk=nblk, T=T, offs=offs, NT=NT, e_core=e_core,
                hidx_l=hidx_l, dstloc_l=dstloc_l, rbfrhs=rbfrhs,
                rbf_lhsT=rbf_lhsT, invcnt=invcnt, nfT=nfT, an_idx=an_idx,
                bslot=bslot, bbase=bbase, bcnt=bcnt, iota=iota)


def _bf(x):
    return np.ascontiguousarray(np.asarray(x, np.float32).astype(ml_dtypes.bfloat16))


def _f32(x):
    return np.ascontiguousarray(np.asarray(x, np.float32))


# --------------------------------------------------------------------------
def _build_program(meta, wflags):
    """Build the SPMD Bass program. `meta` carries the block structure; wflags
    says which bias/scale fast-paths apply (all-zero biases / unit ln_g)."""
    npc, nblk = meta['npc'], meta['nblk']
    T, offs, NT = meta['T'], meta['offs'], meta['NT']
    e_core = meta['e_core']
    Tmax = int(T.max())
    NLAYERS = 3

    nc = bass.Bass(num_devices=NCORES)

    # ---- I/O
    din = {}
    def ein(name, shape, dtype):
        din[name] = nc.dram_tensor(name, shape, dtype, kind='ExternalInput')
        return din[name]

    hidx_in = ein('hidx', [128, NT], I32)
    dstloc_in = ein('dstloc', [128, NT], F32)
    rbfrhs_in = ein('rbfrhs', [4, e_core], F32)
    nfT_in = ein('nfT', [NRBF, npc], F32)
    an_in = ein('an_idx', [128, nblk], I32)
    bslot_in = ein('bslot', [128, nblk], F32)
    invcnt_in = ein('invcnt', [128, nblk], F32)
    iota_in = ein('iota', [128, 128], BF16)
    eyeb_in = ein('eye_bf', [128, 128], BF16)
    eyef_in = ein('eye_f32', [128, 128], F32)
    rbflhsT_in = ein('rbf_lhsT', [4, NRBF], F32)
    embed_in = ein('embed_w', [100, HID], F32)
    projw_in = ein('proj_w', [NRBF, HID], F32)
    fw1_in = ein('fw1b', [NLAYERS, NRBF, HID], BF16)
    fw2_in = ein('fw2b', [NLAYERS, HID, HID], BF16)
    fw3_in = ein('fw3b', [NLAYERS, HID, HID], BF16)
    uw1t_in = ein('uw1t', [NLAYERS, HID, HID], BF16)
    uw1b_in = ein('uw1b', [NLAYERS, HID, HID], BF16)
    uw2_in = ein('uw2b', [NLAYERS, HID, HID], BF16)
    esw1_in = ein('esw1', [HID, HID], BF16)
    bdiag_in = ein('bdiag', [HID, 8], BF16)
    # per-partition bias columns (free to apply)
    fb1_in = ein('fb1c', [HID, NLAYERS], F32)
    fb2_in = ein('fb2c', [HID, NLAYERS], F32)
    ub1_in = ein('ub1c', [HID, NLAYERS], F32)
    ub2_in = ein('ub2c', [HID, NLAYERS], F32)
    b1_in = ein('b1c', [HID, 1], F32)
    # free-dim bias tiles (only used when nonzero)
    fb3_in = ein('fb3t', [NLAYERS, 128, HID], F32)
    projb_in = ein('projbt', [128, HID], F32)
    b2_in = ein('b2t', [128, 8], F32)
    lng_in = ein('lngt', [NLAYERS, 128, HID], F32)
    lnb_in = ein('lnbt', [NLAYERS, 128, HID], F32)

    h_out = nc.dram_tensor('h_out', [npc, HID], F32, kind='ExternalOutput')
    part_out = nc.dram_tensor('part_out', [128, 8], F32, kind='ExternalOutput')
    if DEBUG_TAPS:
        dbg_h0 = nc.dram_tensor('dbg_h0', [128, npc], F32, kind='ExternalOutput')
        dbg_ag = nc.dram_tensor('dbg_ag', [NCORES * npc, HID], F32, kind='ExternalOutput')
        dbg_hsrc = nc.dram_tensor('dbg_hsrc', [128, int(T[0]) * 128], F32, kind='ExternalOutput')
        dbg_agg = nc.dram_tensor('dbg_agg', [128, npc], F32, kind='ExternalOutput')
        dbg_emb = nc.dram_tensor('dbg_emb', [128, npc], F32, kind='ExternalOutput')

    with tile.TileContext(nc) as tc:
        with tc.tile_pool(name='const', bufs=1) as cp, \
             tc.tile_pool(name='slab', bufs=1) as slab, \
             tc.tile_pool(name='gat', bufs=BUFS1) as gat, \
             tc.tile_pool(name='rrp', bufs=BUFS1) as rrp, \
             tc.tile_pool(name='zp', bufs=BUFS2) as zp, \
             tc.tile_pool(name='mp', bufs=BUFS3) as mp, \
             tc.tile_pool(name='np_', bufs=BUFS3) as np_, \
             tc.tile_pool(name='stp', bufs=BUFS4) as stp, \
             tc.tile_pool(name='ppbig', bufs=PBUFS3, space='PSUM') as ppbig, \
             tc.tile_pool(name='ppw', bufs=PBUFS2, space='PSUM') as ppw, \
             tc.tile_pool(name='ppagg', bufs=PBUFS2A, space='PSUM') as ppagg, \
             tc.tile_pool(name='ppart', bufs=1, space='PSUM') as ppart, \
             tc.tile_pool(name='dr', bufs=1, space='DRAM') as dr:

            # ---- load constants / weights to SBUF
            def load(inp, shape, dtype, name):
                t = cp.tile(shape, dtype, name=name)
                nc.sync.dma_start(out=t[:], in_=inp[:])
                return t

            hidx_t = load(hidx_in, [128, NT], I32, 'hidx_t')
            dstloc_t = load(dstloc_in, [128, NT], F32, 'dstloc_t')
            nfT_t = load(nfT_in, [NRBF, npc], F32, 'nfT_t')
            an_t = load(an_in, [128, nblk], I32, 'an_t')
            bslot_t = load(bslot_in, [128, nblk], F32, 'bslot_t')
            invcnt_t = load(invcnt_in, [128, nblk], F32, 'invcnt_t')
            iota_t = load(iota_in, [128, 128], BF16, 'iota_t')
            eyeb_t = load(eyeb_in, [128, 128], BF16, 'eyeb_t')
            eyef_t = load(eyef_in, [128, 128], F32, 'eyef_t')
            rbflhsT_t = load(rbflhsT_in, [4, NRBF], F32, 'rbflhsT_t')
            projw_t = load(projw_in, [NRBF, HID], F32, 'projw_t')
            esw1_t = load(esw1_in, [HID, HID], BF16, 'esw1_t')
            bdiag_t = load(bdiag_in, [HID, 8], BF16, 'bdiag_t')
            fb1_t = load(fb1_in, [HID, NLAYERS], F32, 'fb1_t')
            fb2_t = load(fb2_in, [HID, NLAYERS], F32, 'fb2_t')
            ub1_t = load(ub1_in, [HID, NLAYERS], F32, 'ub1_t')
            ub2_t = load(ub2_in, [HID, NLAYERS], F32, 'ub2_t')
            b1_t = load(b1_in, [HID, 1], F32, 'b1_t')
            fw1_t = [load(fw1_in[l], [NRBF, HID], BF16, f'fw1_{l}') for l in range(NLAYERS)]
            fw2_t = [load(fw2_in[l], [HID, HID], BF16, f'fw2_{l}') for l in range(NLAYERS)]
            fw3_t = [load(fw3_in[l], [HID, HID], BF16, f'fw3_{l}') for l in range(NLAYERS)]
            uw1t_t = [load(uw1t_in[l], [HID, HID], BF16, f'uw1t_{l}') for l in range(NLAYERS)]
            uw1b_t = [load(uw1b_in[l], [HID, HID], BF16, f'uw1b_{l}') for l in range(NLAYERS)]
            uw2_t = [load(uw2_in[l], [HID, HID], BF16, f'uw2_{l}') for l in range(NLAYERS)]
            fb3_t = projb_t = b2_t = None
            if wflags['fb3']:
                fb3_t = [load(fb3_in[l], [128, HID], F32, f'fb3_{l}') for l in range(NLAYERS)]
            if wflags['projb']:
                projb_t = load(projb_in, [128, HID], F32, 'projb_t')
            if wflags['b2']:
                b2_t = load(b2_in, [128, 8], F32, 'b2_t')
            lng_t = lnb_t = None
            if wflags['lng']:
                lng_t = [load(lng_in[l], [128, HID], F32, f'lng_{l}') for l in range(NLAYERS)]
            if wflags['lnb']:
                lnb_t = [load(lnb_in[l], [128, HID], F32, f'lnb_{l}') for l in range(NLAYERS)]
            eps_t = cp.tile([128, 1], F32, name='eps_t')
            nc.vector.memset(eps_t[:], LN_EPS)

            # persistent slabs
            h_sb = slab.tile([128, npc], F32, name='h_sb')         # node-major
            hT_sb = slab.tile([128, npc], BF16, name='hT_sb')       # feat-major
            aggT_sb = slab.tile([128, npc], BF16, name='aggT_sb')   # feat-major

            # collective buffers (one pair per AllGather — Shared DRAM tiles
            # only admit a single writer)
            cc_ins = [dr.tile([npc, HID], F32, kind='Internal', name=f'cc_in{l}')
                      for l in range(NLAYERS)]
            cc_outs = [dr.tile([NCORES * npc, HID], F32, kind='Internal',
                               addr_space='Shared', name=f'cc_out{l}')
                       for l in range(NLAYERS)]

            # ---- h0 = embed_w[an] + nf @ proj_w (+ proj_b)
            emb_t = cp.tile([128, nblk * HID], F32, name='emb_t')
            for j in range(nblk):
                nc.gpsimd.indirect_dma_start(
                    out=emb_t[:, j * HID:(j + 1) * HID], out_offset=None,
                    in_=embed_in[:],
                    in_offset=bass.IndirectOffsetOnAxis(ap=an_t[:, j:j + 1], axis=0))
            for j in range(nblk):
                ps = ppw.tile([128, HID], F32, tag='w', name=f'ps_h0_{j}')
                nc.tensor.matmul(ps[:], lhsT=nfT_t[:, j * 128:(j + 1) * 128],
                                 rhs=projw_t[:], start=True, stop=True)
                sl = h_sb[:, j * HID:(j + 1) * HID]
                nc.vector.tensor_tensor(out=sl, in0=ps[:],
                                        in1=emb_t[:, j * HID:(j + 1) * HID],
                                        op=mybir.AluOpType.add)
                if projb_t is not None:
                    nc.vector.tensor_tensor(out=sl, in0=sl, in1=projb_t[:],
                                            op=mybir.AluOpType.add)

            h_sb_v = h_sb[:].rearrange("p (j f) -> p j f", f=HID)
            nc.sync.dma_start(
                out=cc_ins[0][:].rearrange("(j p) f -> p j f", p=128), in_=h_sb_v)
            if FAKE_CC:
                nc.sync.dma_start(out=cc_outs[0][:npc, :], in_=cc_ins[0][:])
            else:
                nc.gpsimd.collective_compute(
                    'AllGather', mybir.AluOpType.bypass,
                    replica_groups=[list(range(NCORES))],
                    ins=[cc_ins[0][:]], outs=[cc_outs[0][:]])

            if DEBUG_TAPS:
                nc.sync.dma_start(out=dbg_emb[:], in_=emb_t[:])
                nc.sync.dma_start(out=dbg_h0[:], in_=h_sb[:])
                agv = cc_outs[0][:].rearrange("(j p) f -> p j f", p=128)
                agt = cp.tile([128, NCORES * npc // 128, HID], F32, name='agt')
                nc.sync.dma_start(out=agt[:], in_=agv)
                nc.sync.dma_start(
                    out=dbg_ag[:].rearrange("(j p) f -> p j f", p=128), in_=agt[:])
            relu = mybir.ActivationFunctionType.Relu
            for l in range(min(NLAYERS, LAYER_LIMIT)):
                # ================= edge phase =================
                for b in range(nblk):
                    tb = int(T[b])
                    ne = tb * 128
                    hsrc = gat.tile([128, Tmax * 128], F32, tag='hsrc',
                                    name=f'hsrc_{l}_{b}')
                    # HW indirect DMA takes ONE row index per partition —
                    # issue one gather per 128-edge tile
                    if ABL_GATHER:
                        nc.sync.dma_start(
                            out=hsrc[:, :ne].rearrange("p (j f) -> p j f", f=128),
                            in_=cc_outs[l][:ne, :].rearrange(
                                "(j p) f -> p j f", p=128))
                    else:
                        for j in range(tb):
                            col = int(offs[b]) + j
                            nc.gpsimd.indirect_dma_start(
                                out=hsrc[:, j * 128:(j + 1) * 128], out_offset=None,
                                in_=cc_outs[l][:],
                                in_offset=bass.IndirectOffsetOnAxis(
                                    ap=hidx_t[:, col:col + 1], axis=0))
                    if DEBUG_TAPS and l == 0 and b == 0:
                        nc.sync.dma_start(out=dbg_hsrc[:], in_=hsrc[:, :ne])
                    rr = rrp.tile([4, Tmax * 128], F32, tag='rr',
                                  name=f'rr_{l}_{b}')
                    nc.sync.dma_start(
                        out=rr[:, :ne],
                        in_=rbfrhs_in[:, int(offs[b]) * 128:int(offs[b]) * 128 + ne])
                    pagg = ppagg.tile([128, HID], F32, tag='agg', name=f'pagg_{l}_{b}')
                    for c0 in range(0, ne, 512):
                        cs = min(512, ne - c0)
                        if ABL_MLP:
                            for t in range(cs // 128):
                                j = c0 // 128 + t
                                msg = mp.tile([128, HID], BF16, tag='msg',
                                              name=f'msgA_{l}_{b}_{j}')
                                nc.vector.tensor_copy(
                                    out=msg[:], in_=hsrc[:, j * 128:(j + 1) * 128])
                                if ABL_SCATTER and j > 0:
                                    continue
                                oh = mp.tile([128, 128], BF16, tag='oh',
                                             name=f'ohA_{l}_{b}_{j}')
                                nc.vector.tensor_scalar(
                                    out=oh[:], in0=iota_t[:],
                                    scalar1=dstloc_t[:, int(offs[b]) + j:int(offs[b]) + j + 1],
                                    scalar2=None, op0=mybir.AluOpType.is_equal)
                                nc.tensor.matmul(
                                    pagg[:], lhsT=oh[:], rhs=msg[:], start=(j == 0),
                                    stop=(j == tb - 1) or ABL_SCATTER)
                            continue
                        prb = ppbig.tile([NRBF, 512], F32, tag='pbig',
                                         name=f'prb_{l}_{b}_{c0}')
                        nc.tensor.matmul(prb[:, :cs], lhsT=rbflhsT_t[:],
                                         rhs=rr[:, c0:c0 + cs], start=True, stop=True)
                        rbf = zp.tile([NRBF, 512], BF16, tag='rbf',
                                      name=f'rbf_{l}_{b}_{c0}')
                        nc.scalar.activation(out=rbf[:, :cs], in_=prb[:, :cs],
                                             func=mybir.ActivationFunctionType.Exp)
                        pz1 = ppbig.tile([128, 512], F32, tag='pbig',
                                         name=f'pz1_{l}_{b}_{c0}')
                        nc.tensor.matmul(pz1[:, :cs], lhsT=fw1_t[l][:],
                                         rhs=rbf[:, :cs], start=True, stop=True)
                        z1 = zp.tile([128, 512], BF16, tag='z1',
                                     name=f'z1_{l}_{b}_{c0}')
                        nc.scalar.activation(out=z1[:, :cs], in_=pz1[:, :cs],
                                             func=relu, bias=fb1_t[:, l:l + 1])
                        pz2 = ppbig.tile([128, 512], F32, tag='pbig',
                                         name=f'pz2_{l}_{b}_{c0}')
                        nc.tensor.matmul(pz2[:, :cs], lhsT=fw2_t[l][:],
                                         rhs=z1[:, :cs], start=True, stop=True)
                        z2 = zp.tile([128, 512], BF16, tag='z2',
                                     name=f'z2_{l}_{b}_{c0}')
                        nc.vector.tensor_scalar(out=z2[:, :cs], in0=pz2[:, :cs],
                                                scalar1=fb2_t[:, l:l + 1], scalar2=0.0,
                                                op0=mybir.AluOpType.add,
                                                op1=mybir.AluOpType.max)
                        for t in range(cs // 128):
                            j = c0 // 128 + t
                            pw = ppw.tile([128, HID], F32, tag='w',
                                          name=f'pw_{l}_{b}_{j}')
                            nc.tensor.matmul(pw[:], lhsT=z2[:, t * 128:(t + 1) * 128],
                                             rhs=fw3_t[l][:], start=True, stop=True)
                            msg = mp.tile([128, HID], BF16, tag='msg',
                                          name=f'msg_{l}_{b}_{j}')
                            if fb3_t is not None:
                                wsb = mp.tile([128, HID], F32, tag='wsb',
                                              name=f'wsb_{l}_{b}_{j}')
                                nc.vector.tensor_tensor(out=wsb[:], in0=pw[:],
                                                        in1=fb3_t[l][:],
                                                        op=mybir.AluOpType.add)
                                nc.vector.tensor_tensor(
                                    out=msg[:], in0=wsb[:],
                                    in1=hsrc[:, j * 128:(j + 1) * 128],
                                    op=mybir.AluOpType.mult)
                            else:
                                nc.vector.tensor_tensor(
                                    out=msg[:], in0=pw[:],
                                    in1=hsrc[:, j * 128:(j + 1) * 128],
                                    op=mybir.AluOpType.mult)
                            if ABL_SCATTER and j > 0:
                                continue
                            oh = mp.tile([128, 128], BF16, tag='oh',
                                         name=f'oh_{l}_{b}_{j}')
                            nc.vector.tensor_scalar(
                                out=oh[:], in0=iota_t[:],
                                scalar1=dstloc_t[:, int(offs[b]) + j:int(offs[b]) + j + 1],
                                scalar2=None, op0=mybir.AluOpType.is_equal)
                            nc.tensor.matmul(
                                pagg[:], lhsT=oh[:], rhs=msg[:], start=(j == 0),
                                stop=(j == tb - 1) or ABL_SCATTER)
                    # agg evac: scale by 1/cnt, transpose into aggT slab
                    aggb = mp.tile([128, HID], BF16, tag='aggb', name=f'aggb_{l}_{b}')
                    nc.vector.tensor_scalar(out=aggb[:], in0=pagg[:],
                                            scalar1=invcnt_t[:, b:b + 1], scalar2=None,
                                            op0=mybir.AluOpType.mult)
                    if DEBUG_TAPS and l == 0:
                        aggf = mp.tile([128, HID], F32, tag='aggf', name=f'aggf_{b}')
                        nc.vector.tensor_copy(out=aggf[:], in_=aggb[:])
                        nc.sync.dma_start(
                            out=dbg_agg[:, b * HID:(b + 1) * HID], in_=aggf[:])
                    pt = ppw.tile([128, HID], BF16, tag='w', name=f'paggT_{l}_{b}')
                    nc.tensor.transpose(pt[:], aggb[:], eyeb_t[:])
                    nc.vector.tensor_copy(out=aggT_sb[:, b * 128:(b + 1) * 128],
                                          in_=pt[:])

                # ================= node phase =================
                for j in range(nblk if not ABL_NODE else 0):
                    ph = ppw.tile([128, HID], F32, tag='w', name=f'phT_{l}_{j}')
                    nc.tensor.transpose(ph[:], h_sb[:, j * 128:(j + 1) * 128],
                                        eyef_t[:])
                    nc.vector.tensor_copy(out=hT_sb[:, j * 128:(j + 1) * 128],
                                          in_=ph[:])
                for k in range(npc // 512 if not ABL_NODE else 0):
                    sl = slice(k * 512, (k + 1) * 512)
                    pu1 = ppbig.tile([128, 512], F32, tag='pbig', name=f'pu1_{l}_{k}')
                    nc.tensor.matmul(pu1[:], lhsT=uw1t_t[l][:], rhs=hT_sb[:, sl],
                                     start=True, stop=False)
                    nc.tensor.matmul(pu1[:], lhsT=uw1b_t[l][:], rhs=aggT_sb[:, sl],
                                     start=False, stop=True)
                    u1 = zp.tile([128, 512], BF16, tag='u1', name=f'u1_{l}_{k}')
                    nc.scalar.activation(out=u1[:], in_=pu1[:], func=relu,
                                         bias=ub1_t[:, l:l + 1])
                    pupd = ppbig.tile([128, 512], F32, tag='pbig', name=f'pupd_{l}_{k}')
                    nc.tensor.matmul(pupd[:], lhsT=uw2_t[l][:], rhs=u1[:],
                                     start=True, stop=True)
                    updT = zp.tile([128, 512], F32, tag='updT', name=f'updT_{l}_{k}')
                    nc.vector.tensor_scalar(out=updT[:], in0=pupd[:],
                                            scalar1=ub2_t[:, l:l + 1], scalar2=None,
                                            op0=mybir.AluOpType.add)
                    for t in range(4):
                        j = k * 4 + t
                        pn = ppw.tile([128, HID], F32, tag='w', name=f'pn_{l}_{j}')
                        nc.tensor.transpose(pn[:], updT[:, t * 128:(t + 1) * 128],
                                            eyef_t[:])
                        un = np_.tile([128, HID], F32, tag='un', name=f'un_{l}_{j}')
                        nc.vector.tensor_copy(out=un[:], in_=pn[:])
                        st = stp.tile([128, 6], F32, tag='st', name=f'st_{l}_{j}')
                        nc.vector.bn_stats(out=st[:], in_=un[:])
                        mv = stp.tile([128, 2], F32, tag='mv', name=f'mv_{l}_{j}')
                        nc.vector.bn_aggr(out=mv[:], in_=st[:])
                        sd = stp.tile([128, 1], F32, tag='sd', name=f'sd_{l}_{j}')
                        nc.scalar.activation(out=sd[:], in_=mv[:, 1:2],
                                             func=mybir.ActivationFunctionType.Sqrt,
                                             bias=eps_t[:, :1])
                        rstd = stp.tile([128, 1], F32, tag='rstd', name=f'rstd_{l}_{j}')
                        nc.vector.reciprocal(out=rstd[:], in_=sd[:])
                        nrm = np_.tile([128, HID], F32, tag='nrm', name=f'nrm_{l}_{j}')
                        nc.vector.tensor_scalar(out=nrm[:], in0=un[:],
                                                scalar1=mv[:, 0:1], scalar2=rstd[:, 0:1],
                                                op0=mybir.AluOpType.subtract,
                                                op1=mybir.AluOpType.mult)
                        if lng_t is not None:
                            nc.vector.tensor_tensor(out=nrm[:], in0=nrm[:],
                                                    in1=lng_t[l][:],
                                                    op=mybir.AluOpType.mult)
                        if lnb_t is not None:
                            nc.vector.tensor_tensor(out=nrm[:], in0=nrm[:],
                                                    in1=lnb_t[l][:],
                                                    op=mybir.AluOpType.add)
                        hs = h_sb[:, j * 128:(j + 1) * 128]
                        nc.vector.tensor_tensor(out=hs, in0=nrm[:], in1=hs,
                                                op=mybir.AluOpType.add)
                if l < NLAYERS - 1:
                    nc.sync.dma_start(
                        out=cc_ins[l + 1][:].rearrange("(j p) f -> p j f", p=128),
                        in_=h_sb_v)
                    if FAKE_CC:
                        nc.sync.dma_start(out=cc_outs[l + 1][:npc, :],
                                          in_=cc_ins[l + 1][:])
                    else:
                        nc.gpsimd.collective_compute(
                            'AllGather', mybir.AluOpType.bypass,
                            replica_groups=[list(range(NCORES))],
                            ins=[cc_ins[l + 1][:]], outs=[cc_outs[l + 1][:]])

            # ================= readout =================
            for j in range(nblk):
                ph = ppw.tile([128, HID], F32, tag='w', name=f'phTr_{j}')
                nc.tensor.transpose(ph[:], h_sb[:, j * 128:(j + 1) * 128], eyef_t[:])
                nc.vector.tensor_copy(out=hT_sb[:, j * 128:(j + 1) * 128], in_=ph[:])
            ppart_t = ppart.tile([128, 8], F32, tag='part', name='ppart_t')
            for k in range(npc // 512):
                sl = slice(k * 512, (k + 1) * 512)
                pr1 = ppbig.tile([128, 512], F32, tag='pbig', name=f'pr1_{k}')
                nc.tensor.matmul(pr1[:], lhsT=esw1_t[:], rhs=hT_sb[:, sl],
                                 start=True, stop=True)
                r1 = zp.tile([128, 512], BF16, tag='r1', name=f'r1_{k}')
                nc.scalar.activation(out=r1[:], in_=pr1[:], func=relu,
                                     bias=b1_t[:, :1])
                for t in range(4):
                    j = k * 4 + t
                    pes = ppw.tile([128, 8], F32, tag='w', name=f'pes_{j}')
                    nc.tensor.matmul(pes[:], lhsT=r1[:, t * 128:(t + 1) * 128],
                                     rhs=bdiag_t[:], start=True, stop=True)
                    es = mp.tile([128, 8], BF16, tag='es', name=f'es_{j}')
                    if b2_t is not None:
                        esf = mp.tile([128, 8], F32, tag='esf', name=f'esf_{j}')
                        nc.vector.tensor_tensor(out=esf[:], in0=pes[:], in1=b2_t[:],
                                                op=mybir.AluOpType.add)
                        nc.vector.tensor_copy(out=es[:], in_=esf[:])
                    else:
                        nc.vector.tensor_copy(out=es[:], in_=pes[:])
                    ohb = mp.tile([128, 128], BF16, tag='oh', name=f'ohb_{j}')
                    nc.vector.tensor_scalar(out=ohb[:], in0=iota_t[:],
                                            scalar1=bslot_t[:, j:j + 1], scalar2=None,
                                            op0=mybir.AluOpType.is_equal)
                    nc.tensor.matmul(ppart_t[:], lhsT=ohb[:], rhs=es[:],
                                     start=(j == 0), stop=(j == nblk - 1))
            psb = cp.tile([128, 8], F32, name='psb')
            nc.vector.tensor_copy(out=psb[:], in_=ppart_t[:])
            nc.sync.dma_start(out=part_out[:], in_=psb[:])
            nc.sync.dma_start(out=h_out[:].rearrange("(j p) f -> p j f", p=128),
                              in_=h_sb_v)

    _split_multi_waits(nc)
    return nc


# --------------------------------------------------------------------------
def _make_in_maps(inputs, meta):
    NLAYERS = 3
    wf = {}
    wf['fb3'] = bool(np.abs(np.asarray(inputs['fb3'])).max() > 0)
    wf['projb'] = bool(np.abs(np.asarray(inputs['proj_b'])).max() > 0)
    eb2 = np.asarray(inputs['eb2']); sb2 = np.asarray(inputs['sb2'])
    wf['b2'] = bool(max(np.abs(eb2).max(), np.abs(sb2).max()) > 0)
    wf['lng'] = bool(np.abs(np.asarray(inputs['ln_g']) - 1.0).max() > 0)
    wf['lnb'] = bool(np.abs(np.asarray(inputs['ln_b'])).max() > 0)

    ew1 = _f32(inputs['ew1']); sw1 = _f32(inputs['sw1'])
    ew2 = _f32(inputs['ew2']); sw2 = _f32(inputs['sw2'])
    esw1 = np.concatenate([ew1, sw1], 1)
    bdiag = np.zeros((HID, 8), np.float32)
    bdiag[:64, 0] = ew2[:, 0]
    bdiag[64:, 1:5] = sw2
    b1 = np.concatenate([_f32(inputs['eb1']), _f32(inputs['sb1'])])[:, None]
    b2 = np.zeros((128, 8), np.float32)
    b2[:, 0] = float(np.asarray(inputs['eb2'])[0])
    b2[:, 1:5] = _f32(inputs['sb2'])[None, :]

    shared = {
        'iota': _bf(meta['iota']),
        'eye_bf': _bf(np.eye(128, dtype=np.float32)),
        'eye_f32': _f32(np.eye(128, dtype=np.float32)),
        'rbf_lhsT': _f32(meta['rbf_lhsT']),
        'embed_w': _f32(inputs['embed_w']),
        'proj_w': _f32(inputs['proj_w']),
        'fw1b': _bf(inputs['fw1']),
        'fw2b': _bf(inputs['fw2']),
        'fw3b': _bf(inputs['fw3']),
        'uw1t': _bf(np.asarray(inputs['uw1'], np.float32)[:, :HID, :]),
        'uw1b': _bf(np.asarray(inputs['uw1'], np.float32)[:, HID:, :]),
        'uw2b': _bf(inputs['uw2']),
        'esw1': _bf(esw1),
        'bdiag': _bf(bdiag),
        'fb1c': _f32(inputs['fb1']).T.copy(),
        'fb2c': _f32(inputs['fb2']).T.copy(),
        'ub1c': _f32(inputs['ub1']).T.copy(),
        'ub2c': _f32(inputs['ub2']).T.copy(),
        'b1c': b1,
        'fb3t': _f32(np.broadcast_to(np.asarray(inputs['fb3'], np.float32)[:, None, :],
                                     (NLAYERS, 128, HID)).copy()),
        'projbt': _f32(np.broadcast_to(np.asarray(inputs['proj_b'], np.float32)[None, :],
                                       (128, HID)).copy()),
        'b2t': b2,
        'lngt': _f32(np.broadcast_to(np.asarray(inputs['ln_g'], np.float32)[:, None, :],
                                     (NLAYERS, 128, HID)).copy()),
        'lnbt': _f32(np.broadcast_to(np.asarray(inputs['ln_b'], np.float32)[:, None, :],
                                     (NLAYERS, 128, HID)).copy()),
    }
    in_maps = []
    for c in range(NCORES):
        m = dict(shared)
        m['hidx'] = np.ascontiguousarray(meta['hidx_l'][c])
        m['dstloc'] = np.ascontiguousarray(meta['dstloc_l'][c])
        m['rbfrhs'] = np.ascontiguousarray(meta['rbfrhs'][c])
        m['nfT'] = np.ascontiguousarray(meta['nfT'][c])
        m['an_idx'] = np.ascontiguousarray(meta['an_idx'][c])
        m['bslot'] = np.ascontiguousarray(meta['bslot'][c])
        m['invcnt'] = np.ascontiguousarray(meta['invcnt'][c])
        in_maps.append(m)
    return in_maps, wf


def _combine(meta, results):
    n_batch = meta['n_batch']
    energy = np.zeros(n_batch, np.float64)
    ssum = np.zeros((n_batch, 4), np.float64)
    h_parts = []
    for c in range(NCORES):
        part = results[c]['part_out']
        base = int(meta['bbase'][c])
        hi = min(128, n_batch - base)
        energy[base:base + hi] += part[:hi, 0]
        ssum[base:base + hi] += part[:hi, 1:5]
        h_parts.append(results[c]['h_out'][:meta['npc_real']])
    sel = ssum / np.maximum(meta['bcnt'], 1.0)[:, None]
    h = np.concatenate(h_parts, 0)
    return (energy[:, None].astype(np.float32), sel.astype(np.float32),
            h.astype(np.float32))


def run(inputs, trace=False, **run_kwargs):
    meta = _prep(inputs)
    in_maps, wf = _make_in_maps(inputs, meta)
    nc = _build_program(meta, wf)
    res = run_bass_kernel_spmd(nc, in_maps, core_ids=list(range(NCORES)),
                               trace=trace, **run_kwargs)
    return _combine(meta, res.results), res


def kernel(**inputs):
    out, _ = run(inputs, trace=False)
    return out
